# revision 7
# baseline (speedup 1.0000x reference)
"""Fused transformer block (pre-norm attn + MLP) for Trainium2, 8 cores.

Sharding: data-parallel over batch (32 batches -> 4 per core), no
collectives. Each core computes the full block on its shard.

Per-core dataflow (feature-major activations, tokens in the free dim):
  P1: LN1 stats via ones-matmul (f32r x + bf16 x^2), PE-broadcast of
      rs/mu*rs, DVE apply -> h1T [C, T] bf16; qkT = Wqk^T @ h1T
      (W-stationary); v = h1T^T @ Wv (h1-stationary, batch-aligned token
      tiles) stored token-major with an interleaved ones column per head
      (softmax denominators ride along attn@v as output row 64).
  P2 (per batch): 12 heads of scoresT (kT-stationary, keys-major), exp
      on ACT (no max subtraction: |scores| < 3 for these inputs), oU^T =
      v_ones-stationary @ expT, reciprocal + PE broadcast + DVE mult ->
      o_all; then proj for this batch (K=64 head-tile contraction) +
      residual vs restreamed x; x2 spilled to DRAM.
  P3: LN2 on restreamed x2 -> h2T.
  P4: FC1 + exact-erf Gelu (fused per-partition bias), FC2 + residual.

LN gains/biases and the attention scale are folded into the weights on
the host. Matmul operands bf16 (f32r for the LN sum inputs); fp32
matmuls run 4x slower on the PE. Psum and the residual stream fp32.
LN rsqrt via Ln+Exp so every ACT function before P4 shares one table
set (a table switch costs ~2.7us). SBUF pools are phase-scoped to fit
the 24MB SBUF; the x2 spill round-trips through DRAM.
"""
import numpy as np
import ml_dtypes
import concourse.bacc as bacc
import concourse.mybir as mybir
import concourse.tile as tile
from concourse.bass_utils import run_bass_kernel_spmd

F32 = mybir.dt.float32
F32R = mybir.dt.float32r
BF16 = mybir.dt.bfloat16
AF = mybir.ActivationFunctionType
ALU = mybir.AluOpType

B, N, C = 32, 577, 768
H, D = 12, 64
HID = 3072
NCORES = 8
BPC = B // NCORES            # 4 batches per core
T = BPC * N                  # 2308 tokens per core
CHUNKS = [(0, 512), (512, 512), (1024, 512), (1536, 512), (2048, 260)]
KTILES = [(0, 128), (128, 128), (256, 128), (384, 128), (512, 65)]
QCH = [(0, 512), (512, 65)]  # free-dim split of 577


def _ln_rows(nc, pool, ps_sum, ps_ssq, cw, c0, sfx):
    """Per-token LN stats from psum sums: returns (rs, murs) bf16 [1, cw]."""
    mu = pool.tile([1, cw], F32, name=f"mu{sfx}_{c0}", tag=f"mu{sfx}")
    nc.vector.tensor_scalar(mu[:], ps_sum[:], 1.0 / C, None, ALU.mult)
    t1 = pool.tile([1, cw], F32, name=f"t1{sfx}_{c0}", tag=f"t1{sfx}")
    nc.vector.tensor_tensor(t1[:], ps_sum[:], mu[:], ALU.mult)
    t2 = pool.tile([1, cw], F32, name=f"t2{sfx}_{c0}", tag=f"t2{sfx}")
    nc.vector.tensor_tensor(t2[:], ps_ssq[:], t1[:], ALU.subtract)
    t3 = pool.tile([1, cw], F32, name=f"t3{sfx}_{c0}", tag=f"t3{sfx}")
    nc.vector.tensor_scalar(t3[:], t2[:], 1.0 / C, 1e-5, ALU.mult, ALU.add)
    lnv = pool.tile([1, cw], F32, name=f"lnv{sfx}_{c0}", tag=f"lnv{sfx}")
    nc.scalar.activation(lnv[:], t3[:], AF.Ln)
    rs = pool.tile([1, cw], BF16, name=f"rs{sfx}_{c0}", tag=f"rs{sfx}")
    nc.scalar.activation(rs[:], lnv[:], AF.Exp, scale=-0.5)
    murs = pool.tile([1, cw], BF16, name=f"mr{sfx}_{c0}", tag=f"mr{sfx}")
    nc.vector.tensor_tensor(murs[:], mu[:], rs[:], ALU.mult)
    return rs, murs


def _build_nc():
    nc = bacc.Bacc("TRN2", target_bir_lowering=False, debug=False,
                   num_devices=NCORES)
    xT_d = nc.dram_tensor("xT", [C, T], F32R, kind="ExternalInput")
    wqkv_d = nc.dram_tensor("wqkv", [C, 3 * C], BF16, kind="ExternalInput")
    qkb_d = nc.dram_tensor("qkb", [128, 12], F32, kind="ExternalInput")
    vbb_d = nc.dram_tensor("vbb", [128, C], F32, kind="ExternalInput")
    wp_d = nc.dram_tensor("wp", [12, 64, C], BF16, kind="ExternalInput")
    pb_d = nc.dram_tensor("pb", [128, 6], F32, kind="ExternalInput")
    w1_d = nc.dram_tensor("w1", [C, HID], BF16, kind="ExternalInput")
    b1a_d = nc.dram_tensor("b1a", [128, 24], F32, kind="ExternalInput")
    w2_d = nc.dram_tensor("w2", [HID, C], BF16, kind="ExternalInput")
    b2a_d = nc.dram_tensor("b2a", [128, 6], F32, kind="ExternalInput")
    outT_d = nc.dram_tensor("outT", [C, T], F32, kind="ExternalOutput")

    with tile.TileContext(nc) as tc:
        with tc.tile_pool(name="cst", bufs=1) as cst, \
             tc.tile_pool(name="dram", bufs=1, space="DRAM") as drp:
            ones_bf = cst.tile([128, 1], BF16)
            nc.vector.memset(ones_bf[:], 1.0)
            ones_f = cst.tile([128, 1], F32)
            nc.vector.memset(ones_f[:], 1.0)
            ones_r = cst.tile([128, 1], F32R)
            nc.vector.tensor_copy(ones_r[:], ones_f[:])
            ones_row = cst.tile([1, 128], BF16)
            nc.vector.memset(ones_row[:], 1.0)
            qkb = cst.tile([128, 12], F32)
            nc.sync.dma_start(out=qkb[:], in_=qkb_d[:])
            vbb = cst.tile([128, C], F32)
            nc.sync.dma_start(out=vbb[:], in_=vbb_d[:])
            pb = cst.tile([128, 6], F32)
            nc.sync.dma_start(out=pb[:], in_=pb_d[:])
            b1a = cst.tile([128, 24], F32)
            nc.sync.dma_start(out=b1a[:], in_=b1a_d[:])
            b2a = cst.tile([128, 6], F32)
            nc.sync.dma_start(out=b2a[:], in_=b2a_d[:])
            x2s = drp.tile([C, T], F32, name="x2spill")

            with tc.tile_pool(name="qv", bufs=1) as qv, \
                 tc.tile_pool(name="vbp", bufs=4) as vbp:
                qkT = [qv.tile([128, T], BF16, name=f"qkT{n}") for n in range(12)]
                vbuf = {}
                for b in range(BPC):
                    for i in range(5):
                        vbuf[(b, i)] = vbp.tile([128, H * 65], BF16,
                                                name=f"vb{b}_{i}", tag=f"vb{i}")

                # ---------------- P1: LN1 + qkv + v ----------------
                with tc.tile_pool(name="p1w", bufs=1) as p1w:
                    wq = [p1w.tile([128, 3 * C], BF16, name=f"wq{k}")
                          for k in range(6)]
                    for k in range(6):
                        nc.sync.dma_start(
                            out=wq[k][:], in_=wqkv_d[k * 128:(k + 1) * 128, :])
                    h1 = [p1w.tile([128, T], BF16, name=f"h1_{k}")
                          for k in range(6)]
                    with tc.tile_pool(name="p1t", bufs=2) as p1t, \
                         tc.tile_pool(name="p1s", bufs=1) as p1s, \
                         tc.tile_pool(name="ps1", bufs=1, space="PSUM") as ps1, \
                         tc.tile_pool(name="psqk", bufs=2, space="PSUM") as psqk:
                        for (c0, cw) in CHUNKS:
                            xk = []
                            for k in range(6):
                                xt = p1s.tile([128, cw], F32R, name=f"x{k}_{c0}",
                                              tag=f"x{k}")
                                nc.sync.dma_start(
                                    out=xt[:],
                                    in_=xT_d[k * 128:(k + 1) * 128, c0:c0 + cw])
                                xk.append(xt)
                            ps_sum = ps1.tile([1, cw], F32, name=f"pss_{c0}",
                                              tag="ps_sum")
                            ps_ssq = ps1.tile([1, cw], F32, name=f"psq_{c0}",
                                              tag="ps_ssq")
                            for k in range(6):
                                xq = p1t.tile([128, cw], BF16, name=f"xq{k}_{c0}",
                                              tag=f"xq{k}")
                                nc.vector.tensor_tensor(xq[:], xk[k][:], xk[k][:],
                                                        ALU.mult)
                                nc.tensor.matmul(ps_sum[:], ones_r[:], xk[k][:],
                                                 start=(k == 0), stop=(k == 5))
                                nc.tensor.matmul(ps_ssq[:], ones_bf[:], xq[:],
                                                 start=(k == 0), stop=(k == 5))
                            rs, murs = _ln_rows(nc, p1s, ps_sum, ps_ssq,
                                                cw, c0, "")
                            ps_rs = ps1.tile([128, cw], F32, name=f"prs_{c0}",
                                             tag="ps_rs")
                            nc.tensor.matmul(ps_rs[:], ones_row[:], rs[:],
                                             start=True, stop=True)
                            ps_mu = ps1.tile([128, cw], F32, name=f"pmu_{c0}",
                                             tag="ps_mu")
                            nc.tensor.matmul(ps_mu[:], ones_row[:], murs[:],
                                             start=True, stop=True)
                            for k in range(6):
                                tmp = p1s.tile([128, cw], F32, name=f"tp{k}_{c0}",
                                               tag=f"tp{k}")
                                nc.vector.tensor_tensor(tmp[:], xk[k][:],
                                                        ps_rs[:], ALU.mult)
                                nc.vector.tensor_tensor(h1[k][:, c0:c0 + cw],
                                                        tmp[:], ps_mu[:],
                                                        ALU.subtract)
                            for n in range(12):
                                pq = psqk.tile([128, cw], F32, name=f"pq{n}_{c0}",
                                               tag="psqk")
                                for k in range(6):
                                    nc.tensor.matmul(
                                        pq[:], wq[k][:, n * 128:(n + 1) * 128],
                                        h1[k][:, c0:c0 + cw],
                                        start=(k == 0), stop=(k == 5))
                                nc.scalar.activation(qkT[n][:, c0:c0 + cw],
                                                     pq[:], AF.Identity,
                                                     bias=qkb[:, n:n + 1])
                    # v matmuls: batch-aligned token tiles, h1-stationary
                    with tc.tile_pool(name="psv", bufs=2, space="PSUM") as psv:
                        for b in range(BPC):
                            for i, (kt0, kr) in enumerate(KTILES):
                                m0 = b * N + kt0
                                pv = psv.tile([kr, C], F32, name=f"pv{b}_{i}",
                                              tag="psv")
                                for k in range(6):
                                    nc.tensor.matmul(
                                        pv[:, 0:512], h1[k][:, m0:m0 + kr],
                                        wq[k][:, 1536:2048],
                                        start=(k == 0), stop=(k == 5))
                                    nc.tensor.matmul(
                                        pv[:, 512:768], h1[k][:, m0:m0 + kr],
                                        wq[k][:, 2048:2304],
                                        start=(k == 0), stop=(k == 5))
                                vm = vbuf[(b, i)]
                                ones_ap = vm[:kr].rearrange(
                                    "p (h e) -> p h e", e=65)[:, :, 64]
                                nc.vector.memset(ones_ap, 1.0)
                                for h in range(H):
                                    nc.vector.tensor_tensor(
                                        vm[:kr, h * 65:h * 65 + 64],
                                        pv[:kr, h * 64:(h + 1) * 64],
                                        vbb[:kr, h * 64:(h + 1) * 64], ALU.add)

                # ------- P2: attention + per-batch proj/residual -------
                with tc.tile_pool(name="wpp", bufs=1) as wpp, \
                     tc.tile_pool(name="oal", bufs=2) as oal, \
                     tc.tile_pool(name="p2t", bufs=3) as p2t, \
                     tc.tile_pool(name="pexp", bufs=7) as pexp, \
                     tc.tile_pool(name="p2pr", bufs=1) as p2pr, \
                     tc.tile_pool(name="pss", bufs=1, space="PSUM") as pss, \
                     tc.tile_pool(name="pso", bufs=1, space="PSUM") as pso, \
                     tc.tile_pool(name="psb", bufs=1, space="PSUM") as psb, \
                     tc.tile_pool(name="psp", bufs=2, space="PSUM") as psp:
                    wp_sb = [wpp.tile([64, C], BF16, name=f"wp{i}")
                             for i in range(12)]
                    for i in range(12):
                        nc.sync.dma_start(out=wp_sb[i][:], in_=wp_d[i])
                    for b in range(BPC):
                        base = b * N
                        o_all = [oal.tile([64, N], BF16, name=f"oa{b}_{h}",
                                          tag=f"oa{h}") for h in range(H)]
                        for h in range(H):
                            p0 = (h % 2) * 64
                            q_sl = qkT[h // 2][p0:p0 + 64, base:base + N]
                            k_sl = qkT[6 + h // 2][p0:p0 + 64, base:base + N]
                            exps = []
                            for i, (kt0, kr) in enumerate(KTILES):
                                ps_s = pss.tile([kr, N], F32,
                                                name=f"ss{b}_{h}_{i}",
                                                tag="ps_s")
                                for (qc0, qcw) in QCH:
                                    nc.tensor.matmul(
                                        ps_s[:, qc0:qc0 + qcw],
                                        k_sl[:, kt0:kt0 + kr],
                                        q_sl[:, qc0:qc0 + qcw],
                                        start=True, stop=True)
                                e = pexp.tile([kr, N], BF16,
                                              name=f"e{b}_{h}_{i}", tag="exp")
                                nc.scalar.activation(e[:], ps_s[:], AF.Exp)
                                exps.append((e, kr))
                            ps_o = pso.tile([65, N], F32, name=f"po{b}_{h}",
                                            tag="ps_o")
                            for (qc0, qcw) in QCH:
                                for i, (e, kr) in enumerate(exps):
                                    nc.tensor.matmul(
                                        ps_o[:, qc0:qc0 + qcw],
                                        vbuf[(b, i)][:kr, h * 65:(h + 1) * 65],
                                        e[:kr, qc0:qc0 + qcw],
                                        start=(i == 0), stop=(i == 4))
                            oU = p2t.tile([65, N], F32, name=f"oU{b}_{h}",
                                          tag="oU")
                            nc.scalar.activation(oU[:], ps_o[:], AF.Copy)
                            rec = p2t.tile([1, N], BF16, name=f"rc{b}_{h}",
                                           tag="rec")
                            with nc.allow_low_precision(reason="softmax denom"):
                                nc.vector.reciprocal(rec[:], oU[64:65, :])
                            ps_bc = psb.tile([64, N], F32, name=f"pbc{b}_{h}",
                                             tag="ps_bc")
                            for (qc0, qcw) in QCH:
                                nc.tensor.matmul(ps_bc[:, qc0:qc0 + qcw],
                                                 ones_row[0:1, 0:64],
                                                 rec[:, qc0:qc0 + qcw],
                                                 start=True, stop=True)
                            nc.vector.tensor_tensor(o_all[h][:],
                                                    oU[0:64, :], ps_bc[:],
                                                    ALU.mult)
                        # proj + residual for this batch
                        for (qc0, qcw) in QCH:
                            for n in range(6):
                                pp = psp.tile([128, qcw], F32,
                                              name=f"pp{b}_{n}_{qc0}",
                                              tag="psp")
                                for kh in range(12):
                                    nc.tensor.matmul(
                                        pp[:],
                                        wp_sb[kh][:, n * 128:(n + 1) * 128],
                                        o_all[kh][:, qc0:qc0 + qcw],
                                        start=(kh == 0), stop=(kh == 11))
                                tp = p2pr.tile([128, qcw], F32,
                                               name=f"tpp{b}_{n}_{qc0}",
                                               tag=f"tpp{n}")
                                nc.scalar.activation(tp[:], pp[:], AF.Identity,
                                                     bias=pb[:, n:n + 1])
                                xr = p2pr.tile([128, qcw], F32R,
                                               name=f"xr{b}_{n}_{qc0}",
                                               tag=f"xr{n}")
                                nc.sync.dma_start(
                                    out=xr[:],
                                    in_=xT_d[n * 128:(n + 1) * 128,
                                             base + qc0:base + qc0 + qcw])
                                x2 = p2pr.tile([128, qcw], F32,
                                               name=f"x2_{b}_{n}_{qc0}",
                                               tag=f"x2_{n}")
                                nc.vector.tensor_tensor(x2[:], tp[:], xr[:],
                                                        ALU.add)
                                nc.sync.dma_start(
                                    out=x2s[n * 128:(n + 1) * 128,
                                            base + qc0:base + qc0 + qcw],
                                    in_=x2[:])

            # ---------------- P3: LN2 -> h2 ----------------
            with tc.tile_pool(name="h2p", bufs=1) as h2p:
                h2 = [h2p.tile([128, T], BF16, name=f"h2_{k}") for k in range(6)]
                with tc.tile_pool(name="p3b", bufs=2) as p3b, \
                     tc.tile_pool(name="p3s", bufs=1) as p3s, \
                     tc.tile_pool(name="ps3", bufs=1, space="PSUM") as ps3:
                    for (c0, cw) in CHUNKS:
                        xk2 = []
                        ps_sum = ps3.tile([1, cw], F32, name=f"p2s_{c0}",
                                          tag="ps_sum2")
                        ps_ssq = ps3.tile([1, cw], F32, name=f"p2q_{c0}",
                                          tag="ps_ssq2")
                        for k in range(6):
                            xr2 = p3b.tile([128, cw], F32R, name=f"y{k}_{c0}",
                                           tag=f"y{k}")
                            nc.sync.dma_start(
                                out=xr2[:],
                                in_=x2s[k * 128:(k + 1) * 128,
                                        c0:c0 + cw].bitcast(F32R))
                            xk2.append(xr2)
                            xq2 = p3b.tile([128, cw], BF16, name=f"yq{k}_{c0}",
                                           tag=f"yq{k}")
                            nc.vector.tensor_tensor(xq2[:], xr2[:], xr2[:],
                                                    ALU.mult)
                            nc.tensor.matmul(ps_sum[:], ones_r[:], xr2[:],
                                             start=(k == 0), stop=(k == 5))
                            nc.tensor.matmul(ps_ssq[:], ones_bf[:], xq2[:],
                                             start=(k == 0), stop=(k == 5))
                        rs, murs = _ln_rows(nc, p3s, ps_sum, ps_ssq, cw, c0, "2")
                        ps_rs = ps3.tile([128, cw], F32, name=f"pr2_{c0}",
                                         tag="ps_rs2")
                        nc.tensor.matmul(ps_rs[:], ones_row[:], rs[:],
                                         start=True, stop=True)
                        ps_mu = ps3.tile([128, cw], F32, name=f"pm2_{c0}",
                                         tag="ps_mu2")
                        nc.tensor.matmul(ps_mu[:], ones_row[:], murs[:],
                                         start=True, stop=True)
                        for k in range(6):
                            tmp = p3s.tile([128, cw], F32, name=f"tq{k}_{c0}",
                                           tag=f"tq{k}")
                            nc.vector.tensor_tensor(tmp[:], xk2[k][:],
                                                    ps_rs[:], ALU.mult)
                            nc.vector.tensor_tensor(h2[k][:, c0:c0 + cw],
                                                    tmp[:], ps_mu[:],
                                                    ALU.subtract)

                # ---------------- P4: MLP ----------------
                with tc.tile_pool(name="w12", bufs=1) as w12, \
                     tc.tile_pool(name="p4t", bufs=2) as p4t, \
                     tc.tile_pool(name="pgl", bufs=1) as pgl, \
                     tc.tile_pool(name="ps41", bufs=3, space="PSUM") as ps41, \
                     tc.tile_pool(name="ps42", bufs=2, space="PSUM") as ps42:
                    w1_sb = [w12.tile([128, HID], BF16, name=f"w1_{k}")
                             for k in range(6)]
                    for k in range(6):
                        nc.sync.dma_start(out=w1_sb[k][:],
                                          in_=w1_d[k * 128:(k + 1) * 128, :])
                    w2_sb = [w12.tile([128, C], BF16, name=f"w2_{k}")
                             for k in range(24)]
                    for k in range(24):
                        nc.sync.dma_start(out=w2_sb[k][:],
                                          in_=w2_d[k * 128:(k + 1) * 128, :])
                    for (c0, cw) in CHUNKS:
                        gl = []
                        for n1 in range(24):
                            p1p = ps41.tile([128, cw], F32,
                                            name=f"p41_{n1}_{c0}", tag="ps41")
                            for k in range(6):
                                nc.tensor.matmul(
                                    p1p[:],
                                    w1_sb[k][:, n1 * 128:(n1 + 1) * 128],
                                    h2[k][:, c0:c0 + cw],
                                    start=(k == 0), stop=(k == 5))
                            g = pgl.tile([128, cw], BF16, name=f"gl{n1}_{c0}",
                                         tag=f"gl{n1}")
                            nc.scalar.activation(g[:], p1p[:], AF.Gelu,
                                                 bias=b1a[:, n1:n1 + 1])
                            gl.append(g)
                        for n2 in range(6):
                            p2p = ps42.tile([128, cw], F32,
                                            name=f"p42_{n2}_{c0}", tag="ps42")
                            for k2 in range(24):
                                nc.tensor.matmul(
                                    p2p[:],
                                    w2_sb[k2][:, n2 * 128:(n2 + 1) * 128],
                                    gl[k2][:],
                                    start=(k2 == 0), stop=(k2 == 23))
                            t2o = p4t.tile([128, cw], F32,
                                           name=f"t2o{n2}_{c0}", tag="t2o")
                            nc.scalar.activation(t2o[:], p2p[:], AF.Identity,
                                                 bias=b2a[:, n2:n2 + 1])
                            xr2 = p4t.tile([128, cw], F32,
                                           name=f"x2r{n2}_{c0}",
                                           tag=f"x2r{n2}")
                            nc.sync.dma_start(
                                out=xr2[:],
                                in_=x2s[n2 * 128:(n2 + 1) * 128, c0:c0 + cw])
                            oo = p4t.tile([128, cw], F32, name=f"oo{n2}_{c0}",
                                          tag="oo")
                            nc.vector.tensor_tensor(oo[:], t2o[:], xr2[:],
                                                    ALU.add)
                            nc.sync.dma_start(
                                out=outT_d[n2 * 128:(n2 + 1) * 128,
                                           c0:c0 + cw],
                                in_=oo[:])
    nc.compile()
    return nc


_CACHE = {}


def _prep_shared(inputs):
    f32 = np.float32
    qkv_w = np.asarray(inputs["qkv_w"], f32)
    ln1_g = np.asarray(inputs["ln1_g"], f32)
    ln1_b = np.asarray(inputs["ln1_b"], f32)
    qkv_b = np.asarray(inputs["qkv_b"], f32)
    W = qkv_w * ln1_g[:, None]
    bq = ln1_b @ qkv_w + qkv_b
    W = W.copy()
    W[:, :C] *= 0.125
    bq = bq.copy()
    bq[:C] *= 0.125

    proj_w = np.asarray(inputs["proj_w"], f32)
    fc1_w = np.asarray(inputs["fc1_w"], f32)
    ln2_g = np.asarray(inputs["ln2_g"], f32)
    ln2_b = np.asarray(inputs["ln2_b"], f32)
    fc1_b = np.asarray(inputs["fc1_b"], f32)
    W1 = fc1_w * ln2_g[:, None]
    b1 = ln2_b @ fc1_w + fc1_b
    fc2_w = np.asarray(inputs["fc2_w"], f32)

    bf = ml_dtypes.bfloat16
    return {
        "wqkv": np.ascontiguousarray(W.astype(bf)),
        "qkb": np.ascontiguousarray(bq[:2 * C].reshape(12, 128).T.astype(f32)),
        "vbb": np.ascontiguousarray(np.tile(bq[2 * C:], (128, 1)).astype(f32)),
        "wp": np.ascontiguousarray(proj_w.reshape(12, 64, C).astype(bf)),
        "pb": np.ascontiguousarray(
            np.asarray(inputs["proj_b"], f32).reshape(6, 128).T),
        "w1": np.ascontiguousarray(W1.astype(bf)),
        "b1a": np.ascontiguousarray(b1.reshape(24, 128).T.astype(f32)),
        "w2": np.ascontiguousarray(fc2_w.astype(bf)),
        "b2a": np.ascontiguousarray(
            np.asarray(inputs["fc2_b"], f32).reshape(6, 128).T),
    }


def kernel(**inputs):
    if "nc" not in _CACHE:
        _CACHE["nc"] = _build_nc()
    nc = _CACHE["nc"]
    x = np.asarray(inputs["x"], np.float32)
    shared = _prep_shared(inputs)
    in_maps = []
    for c in range(NCORES):
        xT = np.ascontiguousarray(
            x[c * BPC:(c + 1) * BPC].reshape(T, C).T)
        m = {"xT": xT}
        m.update(shared)
        in_maps.append(m)
    res = run_bass_kernel_spmd(nc, in_maps, list(range(NCORES)))
    out = np.empty((B, N, C), np.float32)
    for c in range(NCORES):
        outT = res.results[c]["outT"]
        out[c * BPC:(c + 1) * BPC] = outT.T.reshape(BPC, N, C)
    return out


# revision 9
# speedup vs baseline: 1.2229x; 1.2229x over previous
"""Fused transformer block (pre-norm attn + MLP) for Trainium2, 8 cores.

Sharding: data-parallel over batch (32 batches -> 4 per core), no
collectives. Each core computes the full block on its shard.

Per-core dataflow (feature-major activations, tokens in the free dim):
  P1: LN1 stats via ones-matmul (f32r x + bf16 x^2), PE-broadcast of
      rs/mu*rs, DVE apply -> h1T [C, T] bf16; qkT = Wqk^T @ h1T
      (W-stationary); v = h1T^T @ Wv (h1-stationary, batch-aligned token
      tiles) stored token-major with an interleaved ones column per head
      (softmax denominators ride along attn@v as output row 64).
  P2 (per batch): 12 heads of scoresT (kT-stationary, keys-major), exp
      on ACT (no max subtraction: |scores| < 3 for these inputs), oU^T =
      v_ones-stationary @ expT, reciprocal + PE broadcast + DVE mult ->
      o_all; then proj for this batch (K=64 head-tile contraction) +
      residual vs restreamed x; x2 spilled to DRAM.
  P3: LN2 on restreamed x2 -> h2T.
  P4: FC1 + exact-erf Gelu (fused per-partition bias), FC2 + residual.

LN gains/biases and the attention scale are folded into the weights on
the host. Matmul operands bf16 (f32r for the LN sum inputs); fp32
matmuls run 4x slower on the PE. Psum and the residual stream fp32.
LN rsqrt via Ln+Exp so every ACT function before P4 shares one table
set (a table switch costs ~2.7us). SBUF pools are phase-scoped to fit
the 24MB SBUF; the x2 spill round-trips through DRAM.
"""
import numpy as np
import ml_dtypes
import concourse.bacc as bacc
import concourse.mybir as mybir
import concourse.tile as tile
from concourse.bass_utils import run_bass_kernel_spmd

F32 = mybir.dt.float32
F32R = mybir.dt.float32r
BF16 = mybir.dt.bfloat16
AF = mybir.ActivationFunctionType
ALU = mybir.AluOpType

B, N, C = 32, 577, 768
H, D = 12, 64
HID = 3072
NCORES = 8
BPC = B // NCORES            # 4 batches per core
T = BPC * N                  # 2308 tokens per core
CHUNKS = [(0, 512), (512, 512), (1024, 512), (1536, 512), (2048, 260)]
KTILES = [(0, 128), (128, 128), (256, 128), (384, 128), (512, 65)]
QCH = [(0, 512), (512, 65)]  # free-dim split of 577


def _ln_rows(nc, pool, ps_sum, ps_ssq, cw, c0, sfx):
    """Per-token LN stats from psum sums: returns (rs, murs) bf16 [1, cw]."""
    mu = pool.tile([1, cw], F32, name=f"mu{sfx}_{c0}", tag=f"mu{sfx}")
    nc.vector.tensor_scalar(mu[:], ps_sum[:], 1.0 / C, None, ALU.mult)
    t1 = pool.tile([1, cw], F32, name=f"t1{sfx}_{c0}", tag=f"t1{sfx}")
    nc.vector.tensor_tensor(t1[:], ps_sum[:], mu[:], ALU.mult)
    t2 = pool.tile([1, cw], F32, name=f"t2{sfx}_{c0}", tag=f"t2{sfx}")
    nc.vector.tensor_tensor(t2[:], ps_ssq[:], t1[:], ALU.subtract)
    t3 = pool.tile([1, cw], F32, name=f"t3{sfx}_{c0}", tag=f"t3{sfx}")
    nc.vector.tensor_scalar(t3[:], t2[:], 1.0 / C, 1e-5, ALU.mult, ALU.add)
    lnv = pool.tile([1, cw], F32, name=f"lnv{sfx}_{c0}", tag=f"lnv{sfx}")
    nc.scalar.activation(lnv[:], t3[:], AF.Ln)
    rs = pool.tile([1, cw], BF16, name=f"rs{sfx}_{c0}", tag=f"rs{sfx}")
    nc.scalar.activation(rs[:], lnv[:], AF.Exp, scale=-0.5)
    murs = pool.tile([1, cw], BF16, name=f"mr{sfx}_{c0}", tag=f"mr{sfx}")
    nc.vector.tensor_tensor(murs[:], mu[:], rs[:], ALU.mult)
    return rs, murs


def _build_nc():
    nc = bacc.Bacc("TRN2", target_bir_lowering=False, debug=False,
                   num_devices=NCORES)
    xT_d = nc.dram_tensor("xT", [C, T], F32R, kind="ExternalInput")
    wqkv_d = nc.dram_tensor("wqkv", [C, 3 * C], BF16, kind="ExternalInput")
    qkb_d = nc.dram_tensor("qkb", [128, 12], F32, kind="ExternalInput")
    vbb_d = nc.dram_tensor("vbb", [128, C], F32, kind="ExternalInput")
    wp_d = nc.dram_tensor("wp", [12, 64, C], BF16, kind="ExternalInput")
    pb_d = nc.dram_tensor("pb", [128, 6], F32, kind="ExternalInput")
    w1_d = nc.dram_tensor("w1", [C, HID], BF16, kind="ExternalInput")
    b1a_d = nc.dram_tensor("b1a", [128, 24], F32, kind="ExternalInput")
    w2_d = nc.dram_tensor("w2", [HID, C], BF16, kind="ExternalInput")
    b2a_d = nc.dram_tensor("b2a", [128, 6], F32, kind="ExternalInput")
    outT_d = nc.dram_tensor("outT", [C, T], F32, kind="ExternalOutput")

    with tile.TileContext(nc) as tc:
        with tc.tile_pool(name="cst", bufs=1) as cst, \
             tc.tile_pool(name="dram", bufs=1, space="DRAM") as drp:
            ones_bf = cst.tile([128, 1], BF16)
            nc.vector.memset(ones_bf[:], 1.0)
            ones_f = cst.tile([128, 1], F32)
            nc.vector.memset(ones_f[:], 1.0)
            ones_r = cst.tile([128, 1], F32R)
            nc.vector.tensor_copy(ones_r[:], ones_f[:])
            ones_row = cst.tile([1, 128], BF16)
            nc.vector.memset(ones_row[:], 1.0)
            qkb = cst.tile([128, 12], F32)
            nc.sync.dma_start(out=qkb[:], in_=qkb_d[:])
            vbb = cst.tile([128, C], F32)
            nc.sync.dma_start(out=vbb[:], in_=vbb_d[:])
            pb = cst.tile([128, 6], F32)
            nc.sync.dma_start(out=pb[:], in_=pb_d[:])
            b1a = cst.tile([128, 24], F32)
            nc.sync.dma_start(out=b1a[:], in_=b1a_d[:])
            b2a = cst.tile([128, 6], F32)
            nc.sync.dma_start(out=b2a[:], in_=b2a_d[:])
            x2s = drp.tile([C, T], F32, name="x2spill")

            with tc.tile_pool(name="qv", bufs=1) as qv, \
                 tc.tile_pool(name="vbp", bufs=4) as vbp:
                qkT = [qv.tile([128, T], BF16, name=f"qkT{n}") for n in range(12)]
                vbuf = {}
                for b in range(BPC):
                    for i in range(5):
                        vbuf[(b, i)] = vbp.tile([128, H * 65], BF16,
                                                name=f"vb{b}_{i}", tag=f"vb{i}")

                # ---------------- P1: LN1 + qkv + v ----------------
                with tc.tile_pool(name="p1w", bufs=1) as p1w:
                    wq = [p1w.tile([128, 3 * C], BF16, name=f"wq{k}")
                          for k in range(6)]
                    for k in range(6):
                        nc.sync.dma_start(
                            out=wq[k][:], in_=wqkv_d[k * 128:(k + 1) * 128, :])
                    h1 = [p1w.tile([128, T], BF16, name=f"h1_{k}")
                          for k in range(6)]
                    with tc.tile_pool(name="p1t", bufs=2) as p1t, \
                         tc.tile_pool(name="p1s", bufs=1) as p1s, \
                         tc.tile_pool(name="ps1", bufs=1, space="PSUM") as ps1, \
                         tc.tile_pool(name="psqk", bufs=2, space="PSUM") as psqk:
                        for (c0, cw) in CHUNKS:
                            xk = []
                            for k in range(6):
                                xt = p1s.tile([128, cw], F32R, name=f"x{k}_{c0}",
                                              tag=f"x{k}")
                                nc.sync.dma_start(
                                    out=xt[:],
                                    in_=xT_d[k * 128:(k + 1) * 128, c0:c0 + cw])
                                xk.append(xt)
                            ps_sum = ps1.tile([1, cw], F32, name=f"pss_{c0}",
                                              tag="ps_sum")
                            ps_ssq = ps1.tile([1, cw], F32, name=f"psq_{c0}",
                                              tag="ps_ssq")
                            for k in range(6):
                                xq = p1t.tile([128, cw], BF16, name=f"xq{k}_{c0}",
                                              tag=f"xq{k}")
                                nc.vector.tensor_tensor(xq[:], xk[k][:], xk[k][:],
                                                        ALU.mult)
                                nc.tensor.matmul(ps_sum[:], ones_r[:], xk[k][:],
                                                 start=(k == 0), stop=(k == 5))
                                nc.tensor.matmul(ps_ssq[:], ones_bf[:], xq[:],
                                                 start=(k == 0), stop=(k == 5))
                            rs, murs = _ln_rows(nc, p1s, ps_sum, ps_ssq,
                                                cw, c0, "")
                            ps_rs = p1t.tile([128, cw], BF16, name=f"prs_{c0}",
                                             tag="ps_rs")
                            nc.gpsimd.partition_broadcast(ps_rs[:], rs[:])
                            ps_mu = p1t.tile([128, cw], BF16, name=f"pmu_{c0}",
                                             tag="ps_mu")
                            nc.gpsimd.partition_broadcast(ps_mu[:], murs[:])
                            for k in range(6):
                                tmp = p1s.tile([128, cw], F32, name=f"tp{k}_{c0}",
                                               tag=f"tp{k}")
                                nc.vector.tensor_tensor(tmp[:], xk[k][:],
                                                        ps_rs[:], ALU.mult)
                                nc.vector.tensor_tensor(h1[k][:, c0:c0 + cw],
                                                        tmp[:], ps_mu[:],
                                                        ALU.subtract)
                            for n in range(12):
                                pq = psqk.tile([128, cw], F32, name=f"pq{n}_{c0}",
                                               tag="psqk")
                                for k in range(6):
                                    nc.tensor.matmul(
                                        pq[:], wq[k][:, n * 128:(n + 1) * 128],
                                        h1[k][:, c0:c0 + cw],
                                        start=(k == 0), stop=(k == 5))
                                nc.scalar.activation(qkT[n][:, c0:c0 + cw],
                                                     pq[:], AF.Identity,
                                                     bias=qkb[:, n:n + 1])
                    # v matmuls: batch-aligned token tiles, h1-stationary
                    with tc.tile_pool(name="psv", bufs=2, space="PSUM") as psv:
                        for b in range(BPC):
                            for i, (kt0, kr) in enumerate(KTILES):
                                m0 = b * N + kt0
                                pv = psv.tile([kr, C], F32, name=f"pv{b}_{i}",
                                              tag="psv")
                                for k in range(6):
                                    nc.tensor.matmul(
                                        pv[:, 0:512], h1[k][:, m0:m0 + kr],
                                        wq[k][:, 1536:2048],
                                        start=(k == 0), stop=(k == 5))
                                    nc.tensor.matmul(
                                        pv[:, 512:768], h1[k][:, m0:m0 + kr],
                                        wq[k][:, 2048:2304],
                                        start=(k == 0), stop=(k == 5))
                                vm = vbuf[(b, i)]
                                ones_ap = vm[:kr].rearrange(
                                    "p (h e) -> p h e", e=65)[:, :, 64]
                                nc.vector.memset(ones_ap, 1.0)
                                for h in range(H):
                                    nc.vector.tensor_tensor(
                                        vm[:kr, h * 65:h * 65 + 64],
                                        pv[:kr, h * 64:(h + 1) * 64],
                                        vbb[:kr, h * 64:(h + 1) * 64], ALU.add)

                # ------- P2: attention + per-batch proj/residual -------
                # Heads are software-pipelined: scores/exp for head i+1 are
                # emitted before the attn@v/normalize tail of head i, so the
                # PE never drains while ACT runs exp (keeps HAM at 2.4GHz).
                with tc.tile_pool(name="wpp", bufs=1) as wpp, \
                     tc.tile_pool(name="oal", bufs=2) as oal, \
                     tc.tile_pool(name="p2t", bufs=3) as p2t, \
                     tc.tile_pool(name="pexp", bufs=12) as pexp, \
                     tc.tile_pool(name="p2pr", bufs=1) as p2pr, \
                     tc.tile_pool(name="pss", bufs=2, space="PSUM") as pss, \
                     tc.tile_pool(name="pso", bufs=1, space="PSUM") as pso, \
                     tc.tile_pool(name="psp", bufs=2, space="PSUM") as psp:
                    wp_sb = [wpp.tile([64, C], BF16, name=f"wp{i}")
                             for i in range(12)]
                    for i in range(12):
                        nc.sync.dma_start(out=wp_sb[i][:], in_=wp_d[i])

                    def emit_head(b, h):
                        base = b * N
                        p0 = (h % 2) * 64
                        q_sl = qkT[h // 2][p0:p0 + 64, base:base + N]
                        k_sl = qkT[6 + h // 2][p0:p0 + 64, base:base + N]
                        exps = []
                        for i, (kt0, kr) in enumerate(KTILES):
                            ps_s = pss.tile([kr, N], F32,
                                            name=f"ss{b}_{h}_{i}", tag="ps_s")
                            for (qc0, qcw) in QCH:
                                nc.tensor.matmul(
                                    ps_s[:, qc0:qc0 + qcw],
                                    k_sl[:, kt0:kt0 + kr],
                                    q_sl[:, qc0:qc0 + qcw],
                                    start=True, stop=True)
                            e = pexp.tile([kr, N], BF16,
                                          name=f"e{b}_{h}_{i}", tag="exp")
                            nc.scalar.activation(e[:], ps_s[:], AF.Exp)
                            exps.append((e, kr))
                        return exps

                    def emit_tail(b, h, exps, o_b):
                        ps_o = pso.tile([65, N], F32, name=f"po{b}_{h}",
                                        tag="ps_o")
                        for (qc0, qcw) in QCH:
                            for i, (e, kr) in enumerate(exps):
                                nc.tensor.matmul(
                                    ps_o[:, qc0:qc0 + qcw],
                                    vbuf[(b, i)][:kr, h * 65:(h + 1) * 65],
                                    e[:kr, qc0:qc0 + qcw],
                                    start=(i == 0), stop=(i == 4))
                        oU = p2t.tile([65, N], F32, name=f"oU{b}_{h}",
                                      tag="oU")
                        nc.vector.tensor_copy(oU[:], ps_o[:])
                        rec = p2t.tile([1, N], BF16, name=f"rc{b}_{h}",
                                       tag="rec")
                        with nc.allow_low_precision(reason="softmax denom"):
                            nc.vector.reciprocal(rec[:], oU[64:65, :])
                        bc = p2t.tile([64, N], BF16, name=f"bc{b}_{h}",
                                      tag="bc")
                        nc.gpsimd.partition_broadcast(bc[:], rec[:])
                        nc.vector.tensor_tensor(o_b[h][:], oU[0:64, :],
                                                bc[:], ALU.mult)

                    def emit_proj(b, o_b):
                        base = b * N
                        for (qc0, qcw) in QCH:
                            for n in range(6):
                                pp = psp.tile([128, qcw], F32,
                                              name=f"pp{b}_{n}_{qc0}",
                                              tag="psp")
                                for kh in range(12):
                                    nc.tensor.matmul(
                                        pp[:],
                                        wp_sb[kh][:, n * 128:(n + 1) * 128],
                                        o_b[kh][:, qc0:qc0 + qcw],
                                        start=(kh == 0), stop=(kh == 11))
                                tp = p2pr.tile([128, qcw], F32,
                                               name=f"tpp{b}_{n}_{qc0}",
                                               tag=f"tpp{n}")
                                nc.vector.tensor_scalar(
                                    tp[:], pp[:], pb[:, n:n + 1], None, ALU.add)
                                xr = p2pr.tile([128, qcw], F32R,
                                               name=f"xr{b}_{n}_{qc0}",
                                               tag=f"xr{n}")
                                nc.sync.dma_start(
                                    out=xr[:],
                                    in_=xT_d[n * 128:(n + 1) * 128,
                                             base + qc0:base + qc0 + qcw])
                                x2 = p2pr.tile([128, qcw], F32,
                                               name=f"x2_{b}_{n}_{qc0}",
                                               tag=f"x2_{n}")
                                nc.vector.tensor_tensor(x2[:], tp[:], xr[:],
                                                        ALU.add)
                                nc.sync.dma_start(
                                    out=x2s[n * 128:(n + 1) * 128,
                                            base + qc0:base + qc0 + qcw],
                                    in_=x2[:])

                    o_tiles = {}
                    pending = None
                    for b in range(BPC):
                        o_tiles[b] = [oal.tile([64, N], BF16,
                                               name=f"oa{b}_{h}", tag=f"oa{h}")
                                      for h in range(H)]
                        for h in range(H):
                            exps = emit_head(b, h)
                            if pending is not None:
                                pb_, ph_, pe_ = pending
                                emit_tail(pb_, ph_, pe_, o_tiles[pb_])
                                if ph_ == H - 1:
                                    emit_proj(pb_, o_tiles[pb_])
                            pending = (b, h, exps)
                    pb_, ph_, pe_ = pending
                    emit_tail(pb_, ph_, pe_, o_tiles[pb_])
                    emit_proj(pb_, o_tiles[pb_])

            # ---------------- P3: LN2 -> h2 ----------------
            with tc.tile_pool(name="h2p", bufs=1) as h2p:
                h2 = [h2p.tile([128, T], BF16, name=f"h2_{k}") for k in range(6)]
                with tc.tile_pool(name="p3b", bufs=2) as p3b, \
                     tc.tile_pool(name="p3s", bufs=1) as p3s, \
                     tc.tile_pool(name="ps3", bufs=1, space="PSUM") as ps3:
                    for (c0, cw) in CHUNKS:
                        xk2 = []
                        ps_sum = ps3.tile([1, cw], F32, name=f"p2s_{c0}",
                                          tag="ps_sum2")
                        ps_ssq = ps3.tile([1, cw], F32, name=f"p2q_{c0}",
                                          tag="ps_ssq2")
                        for k in range(6):
                            xr2 = p3b.tile([128, cw], F32R, name=f"y{k}_{c0}",
                                           tag=f"y{k}")
                            nc.sync.dma_start(
                                out=xr2[:],
                                in_=x2s[k * 128:(k + 1) * 128,
                                        c0:c0 + cw].bitcast(F32R))
                            xk2.append(xr2)
                            xq2 = p3b.tile([128, cw], BF16, name=f"yq{k}_{c0}",
                                           tag=f"yq{k}")
                            nc.vector.tensor_tensor(xq2[:], xr2[:], xr2[:],
                                                    ALU.mult)
                            nc.tensor.matmul(ps_sum[:], ones_r[:], xr2[:],
                                             start=(k == 0), stop=(k == 5))
                            nc.tensor.matmul(ps_ssq[:], ones_bf[:], xq2[:],
                                             start=(k == 0), stop=(k == 5))
                        rs, murs = _ln_rows(nc, p3s, ps_sum, ps_ssq, cw, c0, "2")
                        ps_rs = p3b.tile([128, cw], BF16, name=f"pr2_{c0}",
                                         tag="ps_rs2")
                        nc.gpsimd.partition_broadcast(ps_rs[:], rs[:])
                        ps_mu = p3b.tile([128, cw], BF16, name=f"pm2_{c0}",
                                         tag="ps_mu2")
                        nc.gpsimd.partition_broadcast(ps_mu[:], murs[:])
                        for k in range(6):
                            tmp = p3s.tile([128, cw], F32, name=f"tq{k}_{c0}",
                                           tag=f"tq{k}")
                            nc.vector.tensor_tensor(tmp[:], xk2[k][:],
                                                    ps_rs[:], ALU.mult)
                            nc.vector.tensor_tensor(h2[k][:, c0:c0 + cw],
                                                    tmp[:], ps_mu[:],
                                                    ALU.subtract)

                # ---------------- P4: MLP ----------------
                with tc.tile_pool(name="w12", bufs=1) as w12, \
                     tc.tile_pool(name="p4t", bufs=2) as p4t, \
                     tc.tile_pool(name="pgl", bufs=1) as pgl, \
                     tc.tile_pool(name="ps41", bufs=3, space="PSUM") as ps41, \
                     tc.tile_pool(name="ps42", bufs=2, space="PSUM") as ps42:
                    w1_sb = [w12.tile([128, HID], BF16, name=f"w1_{k}")
                             for k in range(6)]
                    for k in range(6):
                        nc.sync.dma_start(out=w1_sb[k][:],
                                          in_=w1_d[k * 128:(k + 1) * 128, :])
                    w2_sb = [w12.tile([128, C], BF16, name=f"w2_{k}")
                             for k in range(24)]
                    for k in range(24):
                        nc.sync.dma_start(out=w2_sb[k][:],
                                          in_=w2_d[k * 128:(k + 1) * 128, :])
                    for (c0, cw) in CHUNKS:
                        gl = []
                        for n1 in range(24):
                            p1p = ps41.tile([128, cw], F32,
                                            name=f"p41_{n1}_{c0}", tag="ps41")
                            for k in range(6):
                                nc.tensor.matmul(
                                    p1p[:],
                                    w1_sb[k][:, n1 * 128:(n1 + 1) * 128],
                                    h2[k][:, c0:c0 + cw],
                                    start=(k == 0), stop=(k == 5))
                            g = pgl.tile([128, cw], BF16, name=f"gl{n1}_{c0}",
                                         tag=f"gl{n1}")
                            nc.scalar.activation(g[:], p1p[:], AF.Gelu,
                                                 bias=b1a[:, n1:n1 + 1])
                            gl.append(g)
                        for n2 in range(6):
                            p2p = ps42.tile([128, cw], F32,
                                            name=f"p42_{n2}_{c0}", tag="ps42")
                            for k2 in range(24):
                                nc.tensor.matmul(
                                    p2p[:],
                                    w2_sb[k2][:, n2 * 128:(n2 + 1) * 128],
                                    gl[k2][:],
                                    start=(k2 == 0), stop=(k2 == 23))
                            t2o = p4t.tile([128, cw], F32,
                                           name=f"t2o{n2}_{c0}", tag="t2o")
                            nc.scalar.activation(t2o[:], p2p[:], AF.Identity,
                                                 bias=b2a[:, n2:n2 + 1])
                            xr2 = p4t.tile([128, cw], F32,
                                           name=f"x2r{n2}_{c0}",
                                           tag=f"x2r{n2}")
                            nc.sync.dma_start(
                                out=xr2[:],
                                in_=x2s[n2 * 128:(n2 + 1) * 128, c0:c0 + cw])
                            oo = p4t.tile([128, cw], F32, name=f"oo{n2}_{c0}",
                                          tag="oo")
                            nc.vector.tensor_tensor(oo[:], t2o[:], xr2[:],
                                                    ALU.add)
                            nc.sync.dma_start(
                                out=outT_d[n2 * 128:(n2 + 1) * 128,
                                           c0:c0 + cw],
                                in_=oo[:])
    nc.compile()
    return nc


_CACHE = {}


def _prep_shared(inputs):
    f32 = np.float32
    qkv_w = np.asarray(inputs["qkv_w"], f32)
    ln1_g = np.asarray(inputs["ln1_g"], f32)
    ln1_b = np.asarray(inputs["ln1_b"], f32)
    qkv_b = np.asarray(inputs["qkv_b"], f32)
    W = qkv_w * ln1_g[:, None]
    bq = ln1_b @ qkv_w + qkv_b
    W = W.copy()
    W[:, :C] *= 0.125
    bq = bq.copy()
    bq[:C] *= 0.125

    proj_w = np.asarray(inputs["proj_w"], f32)
    fc1_w = np.asarray(inputs["fc1_w"], f32)
    ln2_g = np.asarray(inputs["ln2_g"], f32)
    ln2_b = np.asarray(inputs["ln2_b"], f32)
    fc1_b = np.asarray(inputs["fc1_b"], f32)
    W1 = fc1_w * ln2_g[:, None]
    b1 = ln2_b @ fc1_w + fc1_b
    fc2_w = np.asarray(inputs["fc2_w"], f32)

    bf = ml_dtypes.bfloat16
    return {
        "wqkv": np.ascontiguousarray(W.astype(bf)),
        "qkb": np.ascontiguousarray(bq[:2 * C].reshape(12, 128).T.astype(f32)),
        "vbb": np.ascontiguousarray(np.tile(bq[2 * C:], (128, 1)).astype(f32)),
        "wp": np.ascontiguousarray(proj_w.reshape(12, 64, C).astype(bf)),
        "pb": np.ascontiguousarray(
            np.asarray(inputs["proj_b"], f32).reshape(6, 128).T),
        "w1": np.ascontiguousarray(W1.astype(bf)),
        "b1a": np.ascontiguousarray(b1.reshape(24, 128).T.astype(f32)),
        "w2": np.ascontiguousarray(fc2_w.astype(bf)),
        "b2a": np.ascontiguousarray(
            np.asarray(inputs["fc2_b"], f32).reshape(6, 128).T),
    }


def kernel(**inputs):
    if "nc" not in _CACHE:
        _CACHE["nc"] = _build_nc()
    nc = _CACHE["nc"]
    x = np.asarray(inputs["x"], np.float32)
    shared = _prep_shared(inputs)
    in_maps = []
    for c in range(NCORES):
        xT = np.ascontiguousarray(
            x[c * BPC:(c + 1) * BPC].reshape(T, C).T)
        m = {"xT": xT}
        m.update(shared)
        in_maps.append(m)
    res = run_bass_kernel_spmd(nc, in_maps, list(range(NCORES)))
    out = np.empty((B, N, C), np.float32)
    for c in range(NCORES):
        outT = res.results[c]["outT"]
        out[c * BPC:(c + 1) * BPC] = outT.T.reshape(BPC, N, C)
    return out
